# revision 11
# baseline (speedup 1.0000x reference)
"""Head-parallel TRN2 kernel v2 for PVT-style spatial-reduction attention.

Core h owns head h for all 8 batches. Per-core phases:
  A: depthwise 5x5/s2 conv + BN/ReLU + folded 3x3 for OWN batch (PE block-diag)
  B: kv projection for OWN batch (all heads), const-add, k->fp8, v transposed;
     AllToAll redistributes (batch-sharded -> head-sharded), k fp8 / vT bf16
  Q: q projection, head-sharded (all batches), fp8 DoubleRow matmuls; q
     pre-scaled by SCALE*A16 so the S psum slab is a16*(S); DR-interleave via
     a DRAM bounce
  D: per (batch, 128-q-chunk) unit: S matmuls (fp8 DoubleRow) -> slab psum;
     exp via one of 4 paths (class map): ACT exp (+DVE er-mult or PE R-add),
     or fused Schraudolph bits on DVE/Pool (int16 -> bitcast bf16);
     PV with P^T chunk STATIONARY and vaug[k,33] moving -> out [q, 32+den].
  Normalization (divide by den) happens on host.
"""

import os
import sys
from contextlib import ExitStack

sys.path.insert(0, "/opt/trn_rl_repo")

import ml_dtypes
import numpy as np

import concourse.bass as bass
import concourse.mybir as mybir
import concourse.tile as tile
from concourse import bacc
from concourse.bass_utils import run_bass_kernel_spmd

F32 = mybir.dt.float32
F16 = mybir.dt.float16
BF16 = mybir.dt.bfloat16
FP8 = mybir.dt.float8e4
I16 = mybir.dt.int16

B, C, H, W = 8, 256, 56, 56
HEADS, SR, HD = 8, 2, 32
NQ = H * W            # 3136
HK, WK = H // SR, W // SR
NK = HK * WK          # 784
NKP = 896             # NK padded to 7*128
SCALE = HD ** -0.5
QC = 128
N_QC = (NQ + QC - 1) // QC   # 25 (last chunk 64 wide)
KCH = 7
A16 = 128.0 / np.log(2.0)    # schraudolph scale for bf16 bits
B16 = 16256.0 - 4.0          # schraudolph offset
PAYK = 32 * NK               # fp8 bytes of k per (batch, head)
PAYV = 128 * KCH * 32        # fp8 bytes of padded vT per (batch, head)
PAY = PAYK + PAYV

# per-qi class: A = ACT exp + Pool er-mult; P = PE R-add + ACT exp;
# D = DVE fused schraudolph (GPSIMD cannot read PSUM, so no Pool-fused path)
MAP = ['A', 'D', 'P', 'D', 'A', 'D', 'P', 'D', 'A', 'D', 'P', 'D', 'A',
       'D', 'P', 'D', 'P', 'D', 'P', 'D', 'A', 'D', 'A', 'P', 'P']
assert len(MAP) == N_QC


def _qn(qi):
    return min(QC, NQ - qi * QC)


def _offsets(classes):
    """column offset of each qi within the packed table for `classes`."""
    off, out = 0, {}
    for qi in range(N_QC):
        if MAP[qi] in classes:
            out[qi] = off
            off += _qn(qi)
    return out, off


OFF_A, W_A = _offsets(('A',))       # er = exp(R) cols, bf16
OFF_P, W_P = _offsets(('P',))       # a16*R cols, bf16
OFF_DC, W_DC = _offsets(('D',))     # a16*R cols, f16

LAST_RESULTS = None


def _kn(c):
    return 128 if c < KCH - 1 else NK - 128 * (KCH - 1)


def build(nc):
    mult = mybir.AluOpType.mult
    add = mybir.AluOpType.add
    DR = mybir.MatmulPerfMode.DoubleRow

    # ---- DRAM I/O ----
    xp_d = nc.dram_tensor("xp", [C, 60 * 60], FP8, kind="ExternalInput")
    xq_d = nc.dram_tensor("xq", [B, C, NQ], FP8, kind="ExternalInput")
    wq_d = nc.dram_tensor("wqT", [C, 32], BF16, kind="ExternalInput")
    wkv_d = nc.dram_tensor("wkvT", [C, 512], BF16, kind="ExternalInput")
    kvc_d = nc.dram_tensor("kvc", [4, 128, NK], F32, kind="ExternalInput")
    w25_d = nc.dram_tensor("w25d", [C, 25, 128], FP8, kind="ExternalInput")
    w9_d = nc.dram_tensor("w9d", [C, 9, 128], BF16, kind="ExternalInput")
    ab1_d = nc.dram_tensor("ab1", [C, 2], F32, kind="ExternalInput")
    idb_d = nc.dram_tensor("idblk", [128, 32], BF16, kind="ExternalInput")
    idn_d = nc.dram_tensor("idn", [128, 128], BF16, kind="ExternalInput")
    erA_d = nc.dram_tensor("erA", [NKP, W_A], BF16, kind="ExternalInput")
    rpP_d = nc.dram_tensor("rpeP", [NKP, W_P], BF16, kind="ExternalInput")
    rpDC_d = nc.dram_tensor("rpeDC", [NKP, W_DC], F16, kind="ExternalInput")
    out_d = nc.dram_tensor("out", [B, 128, 1024], F32, kind="ExternalOutput")

    # scratch + collective bounce
    qdr_d = nc.dram_tensor("qdr", [2, 128, NQ], FP8)
    a2a_in = nc.dram_tensor("a2a_in", [8, PAY], FP8)
    a2a_out = nc.dram_tensor("a2a_out", [8, PAY], FP8)

    with ExitStack() as ctx:
        tc = ctx.enter_context(tile.TileContext(nc))

        cpool = ctx.enter_context(tc.tile_pool(name="consts", bufs=1))
        wq_t = cpool.tile([128, 2, 32], BF16)
        wkv_t = cpool.tile([128, 2, 4, 128], BF16)
        kvc_t = cpool.tile([128, 4, NK], F32)
        idb_t = cpool.tile([128, 32], BF16)
        idn_t = cpool.tile([128, 128], BF16)
        erA_t = cpool.tile([128, KCH, W_A], BF16)
        rpP_t = cpool.tile([128, KCH, W_P], BF16)
        rpDC_t = cpool.tile([128, KCH, W_DC], F16)
        nc.sync.dma_start(idb_t[:], idb_d.ap())
        # (bulk R-table loads are emitted later, on the ACT queue, so
        # they don't compete with the conv/kv critical path for DMA engines)
        nc.scalar.dma_start(wq_t[:], wq_d.ap().rearrange(
            "(ch p) m -> p ch m", p=128))

        dpool = ctx.enter_context(tc.tile_pool(name="data", bufs=1))
        m_t = dpool.tile([128, 2, NK], BF16)
        kst_t = dpool.tile([128, 2, NK], FP8)     # k staging rows o*128+p
        vst_t = dpool.tile([128, 2, NK], BF16)    # v staging
        vtst_t = dpool.tile([128, KCH, 256], FP8)  # vT staging [k, (h d)]
        qf_t = dpool.tile([128, 2, NQ], FP8)      # q fp8, 4b x 32row layout
        q8_t = dpool.tile([128, 2, 2, NQ], FP8)   # DR layout, 4b x (16+16pad)
        k8_t = dpool.tile([128, 2, 2, NKP], FP8)
        vaug_t = dpool.tile([128, B, KCH, 33], FP8)
        nc.gpsimd.memset(k8_t[:], 0.0)
        nc.gpsimd.memset(vaug_t[:], 0.0)
        nc.gpsimd.memset(vaug_t[:, :, 0:KCH - 1, 32:33], 1.0)
        nc.gpsimd.memset(vaug_t[0:16, :, KCH - 1, 32:33], 1.0)

        xpool = ctx.enter_context(tc.tile_pool(name="xqP", bufs=5))
        xbs_all = []
        for b in range(4):
            xb = xpool.tile([128, 2, NQ], FP8, tag="xb", name=f"xb{b}")
            nc.gpsimd.dma_start(
                xb[:], xq_d.ap()[b].rearrange("(ch p) n -> p ch n", p=128))
            xbs_all.append(xb)

        # ======== Phase A: conv for OWN batch ========
        with tc.tile_pool(name="convA", bufs=1) as apool, \
             tc.tile_pool(name="convPS", bufs=2, space="PSUM") as cps:
            w25_t = apool.tile([64, 2, 2, 25, 128], FP8)
            w9_t = apool.tile([128, 2, 9, 128], BF16)
            ab1_t = apool.tile([128, 2, 2], F32)
            xp_t = apool.tile([64, 2, 2, 60 * 60], FP8)
            tp_t = apool.tile([128, 2, 30 * 30], BF16)
            tmp = apool.tile([128, NK], F32, tag="tmp")
            nc.sync.dma_start(w25_t[:], w25_d.ap().rearrange(
                "(ch p two) t m -> p two ch t m", p=64, two=2))
            nc.sync.dma_start(w9_t[:], w9_d.ap().rearrange(
                "(c p) t m -> p c t m", p=128))
            nc.sync.dma_start(ab1_t[:], ab1_d.ap().rearrange(
                "(c p) m -> p c m", p=128))
            nc.sync.dma_start(
                xp_t[:], xp_d.ap().rearrange(
                    "(ch p two) n -> p two ch n", p=64, two=2))
            nc.sync.dma_start(wkv_t[:], wkv_d.ap().rearrange(
                "(ch p) (o m) -> p ch o m", p=128, m=128))
            nc.sync.dma_start(kvc_t[:], kvc_d.ap().rearrange(
                "o p n -> p o n"))
            nc.gpsimd.memset(tp_t[:], 0.0)

            apss, mpss = [], []
            for ch in range(2):
                x5 = xp_t[:, :, ch, :].rearrange(
                    "p j (h s w t) -> p j h s w t", h=30, s=2, w=30, t=2)
                aps = cps.tile([128, 2, 512], F32, tag="cacc",
                               name=f"aps{ch}")
                apss.append(aps)
                for t in range(25):
                    i, j = divmod(t, 5)
                    qi_, ri = divmod(i, 2)
                    qj, rj = divmod(j, 2)
                    for nh, (r0, r1, nn) in enumerate(
                            ((0, 16, 448), (16, 28, 336))):
                        xv = x5[:, :, qi_ + r0:qi_ + r1, ri,
                                qj:qj + 28, rj]
                        nc.tensor.matmul(
                            aps[:, nh, 0:nn],
                            w25_t[:, :, ch, t, :],
                            xv, start=(t == 0), stop=(t == 24),
                            perf_mode=DR)
            for ch in range(2):
                tp3 = tp_t[:, ch, :].rearrange("p (h w) -> p h w", w=30)
                for nh, (r0, r1, nn) in enumerate(
                        ((0, 16, 448), (16, 28, 336))):
                    nc.vector.tensor_scalar(
                        tmp[:, 0:nn], apss[ch][:, nh, 0:nn],
                        ab1_t[:, ch, 0:1], ab1_t[:, ch, 1:2], mult, add)
                    nc.vector.tensor_scalar_max(
                        tp3[:, 1 + r0:1 + r1, 1:29],
                        tmp[:, 0:nn].rearrange("p (h w) -> p h w", w=28),
                        0.0)
            for ch in range(2):
                tp3 = tp_t[:, ch, :].rearrange("p (h w) -> p h w", w=30)
                mps = cps.tile([128, 2, 512], F32, tag="macc",
                               name=f"mps{ch}")
                mpss.append(mps)
                for t in range(9):
                    i, j = divmod(t, 3)
                    for nh, (r0, r1, nn) in enumerate(
                            ((0, 16, 448), (16, 28, 336))):
                        tpv = tp3[:, i + r0:i + r1, j:j + 28]
                        nc.tensor.matmul(
                            mps[:, nh, 0:nn],
                            w9_t[:, ch, t, :],
                            tpv, start=(t == 0), stop=(t == 8))
            for ch in range(2):
                for nh, (r0, r1, nn) in enumerate(
                        ((0, 16, 448), (16, 28, 336))):
                    nc.vector.tensor_copy(
                        m_t[:, ch, r0 * 28:r0 * 28 + nn],
                        mpss[ch][:, nh, 0:nn])

        # ======== Phase B: kv projection for OWN batch + AllToAll ========
        with tc.tile_pool(name="kvPS", bufs=2, space="PSUM") as kvps, \
             tc.tile_pool(name="vtPS", bufs=2, space="PSUM") as vtps:
            for o in range(4):   # out chunks: k0,k1,v0,v1
                ps = kvps.tile([128, 2, 512], F32, tag="kvp")
                dst = kst_t if o < 2 else vst_t
                for half, (h0, hn) in enumerate(((0, 448), (448, 336))):
                    for ch in range(2):
                        nc.tensor.matmul(
                            ps[:, half, 0:hn],
                            wkv_t[:, ch, o, :],
                            m_t[:, ch, h0:h0 + hn],
                            start=(ch == 0), stop=(ch == 1))
                    nc.vector.tensor_tensor(
                        dst[:, o % 2, h0:h0 + hn],
                        ps[:, half, 0:hn],
                        kvc_t[:, o, h0:h0 + hn], add)
                if o == 1:
                    # k fully staged: ship it while the v path computes
                    for hh in range(HEADS):
                        s2, o2 = hh % 4, hh // 4
                        nc.sync.dma_start(
                            a2a_in.ap()[hh, 0:PAYK].rearrange(
                                "(d n) -> d n", d=32),
                            kst_t[32 * s2:32 * s2 + 32, o2, :])
            # transpose v per (head, kchunk) -> vtst [k, 7, (h*32+d)]
            nc.vector.memset(vtst_t[:, KCH - 1, :], 0.0)
            for hh in range(HEADS):
                s, o = hh % 4, hh // 4
                vt = vtps.tile([128, KCH, 32], BF16, tag="vt")
                for c in range(KCH):
                    kn = _kn(c)
                    nc.tensor.transpose(
                        vt[0:kn, c, :],
                        vst_t[32 * s:32 * s + 32, o,
                              c * 128:c * 128 + kn],
                        idb_t[32 * s:32 * s + 32, :],
                        tile_position=(32 * s, 0))
                nc.scalar.copy(
                    vtst_t[:, 0:KCH - 1, 32 * hh:32 * hh + 32],
                    vt[:, 0:KCH - 1, :])
                nc.scalar.copy(
                    vtst_t[0:16, KCH - 1, 32 * hh:32 * hh + 32],
                    vt[0:16, KCH - 1, :])
            for hh in range(HEADS):
                nc.sync.dma_start(
                    a2a_in.ap()[hh, PAYK:PAY].rearrange(
                        "(c p d) -> p c d", c=KCH, p=128),
                    vtst_t[:, :, 32 * hh:32 * hh + 32])
            nc.gpsimd.collective_compute(
                "AllToAll",
                mybir.AluOpType.bypass,
                replica_groups=[list(range(8))],
                ins=[a2a_in.ap()],
                outs=[a2a_out.ap()],
            )

        # ======== Phases Q + D share one scope so they can overlap ========
        with tc.tile_pool(name="slabPS", bufs=3, space="PSUM") as spool, \
             tc.tile_pool(name="qPS", bufs=1, space="PSUM") as qpps, \
             tc.tile_pool(name="pvPS", bufs=1, space="PSUM") as pvpool, \
             tc.tile_pool(name="ptP", bufs=18) as ptpool, \
             tc.tile_pool(name="obP", bufs=2) as opool:
            # -------- Phase Q: q projection (4-batch col-tiled, fp8 in) ----
            def emit_q(bg):
                if bg == 0:
                    xbs = xbs_all
                else:
                    xbs = []
                    for bi in range(4):
                        b = bg * 4 + bi
                        xb = xpool.tile([128, 2, NQ], FP8, tag="xb")
                        nc.gpsimd.dma_start(
                            xb[:], xq_d.ap()[b].rearrange(
                                "(ch p) n -> p ch n", p=128))
                        xbs.append(xb)
                for nqi in range(7):
                    qps = qpps.tile([128, 448], F32, tag="qps")
                    for bi in range(4):
                        for ch in range(2):
                            nc.tensor.matmul(
                                qps[32 * bi:32 * bi + 32, :],
                                wq_t[:, ch, :],
                                xbs[bi][:, ch,
                                        nqi * 448:(nqi + 1) * 448],
                                start=(ch == 0), stop=(ch == 1),
                                tile_position=(0, 32 * bi))
                    nc.scalar.copy(
                        qf_t[:, bg, nqi * 448:(nqi + 1) * 448], qps[:])
                # bounce through DRAM to build the DR-interleaved layout
                nc.sync.dma_start(qdr_d.ap()[bg], qf_t[:, bg, :])
                for s in range(4):
                    nc.sync.dma_start(
                        q8_t[32 * s:32 * s + 16, bg, :, :],
                        qdr_d.ap()[bg, 32 * s:32 * s + 32, :].rearrange(
                            "(i j) n -> i j n", i=16))

            emit_q(0)

            # R tables + identity, needed only once phase D starts; the
            # gpsimd queue is dammed by the collective until staging is done
            nc.gpsimd.dma_start(idn_t[:], idn_d.ap())
            nc.gpsimd.dma_start(erA_t[:], erA_d.ap().rearrange(
                "(c p) w -> p c w", p=128))
            nc.gpsimd.dma_start(rpP_t[:], rpP_d.ap().rearrange(
                "(c p) w -> p c w", p=128))
            nc.gpsimd.dma_start(rpDC_t[:], rpDC_d.ap().rearrange(
                "(c p) w -> p c w", p=128))

            # k8/vaug loads (these wait on the collective; keep them after
            # the qdr bounce so they don't block the SP queue head)
            for b in range(B):
                bg, s = b // 4, b % 4
                nc.sync.dma_start(
                    k8_t[32 * s:32 * s + 16, bg, :, 0:NK],
                    a2a_out.ap()[b, 0:PAYK].rearrange(
                        "(i j n) -> i j n", i=16, j=2))
            for b in range(B):
                nc.sync.dma_start(
                    vaug_t[:, b, :, 0:32],
                    a2a_out.ap()[b, PAYK:PAY].rearrange(
                        "(c p d) -> p c d", c=KCH, p=128))

            # -------- Phase D: attention units --------
            pending = []
            for b in range(B):
                bg, s = b // 4, b % 4
                pvacc = pvpool.tile([128, 512], F32, tag="pv")
                ob = opool.tile([128, 1024], F32, tag="ob")

                def emit_pv(ent, b_=b, pv_=pvacc, ob_=ob):
                    qi_, qn_, pchunks_ = ent
                    po = (qi_ % 15) * 33
                    for c in range(KCH):
                        nc.tensor.matmul(
                            pv_[0:qn_, po:po + 33],
                            pchunks_[c],
                            vaug_t[:, b_, c, :],
                            start=(c == 0), stop=(c == KCH - 1))
                    # flush pv slots once exhausted (spread the WAR window)
                    if qi_ == 7:
                        nc.scalar.copy(ob_[:, 0:264], pv_[:, 0:264])
                    elif qi_ == 14:
                        nc.scalar.copy(ob_[:, 264:495], pv_[:, 264:495])
                    elif qi_ == 19:
                        nc.vector.tensor_copy(ob_[:, 512:677], pv_[:, 0:165])
                    elif qi_ == N_QC - 1:
                        nc.vector.tensor_copy(ob_[:, 677:842],
                                              pv_[:, 165:330])
                        nc.sync.dma_start(out_d.ap()[b_], ob_[:])

                for qi in range(N_QC):
                    q0, qn = qi * QC, _qn(qi)
                    cls = MAP[qi]
                    slab = spool.tile([128, KCH, QC], F32, tag="slab")
                    for c in range(KCH):
                        nc.tensor.matmul(
                            slab[:, c, 0:qn],
                            k8_t[32 * s:32 * s + 16, bg, :,
                                 c * 128:(c + 1) * 128],
                            q8_t[32 * s:32 * s + 16, bg, :, q0:q0 + qn],
                            start=True, stop=(cls != 'P'),
                            tile_position=(32 * s, 0),
                            perf_mode=DR)
                    if cls == 'P':
                        off = OFF_P[qi]
                        for c in range(KCH):
                            nc.tensor.matmul(
                                slab[:, c, 0:qn],
                                idn_t[:],
                                rpP_t[:, c, off:off + qn],
                                start=False, stop=True)
                    if cls in ('A', 'P'):
                        pt = ptpool.tile([128, KCH, QC], BF16, tag="pt")
                        nc.scalar.activation(
                            pt[:, :, 0:qn], slab[:, :, 0:qn],
                            mybir.ActivationFunctionType.Exp,
                            scale=float(1.0 / A16))
                        if cls == 'A':
                            off = OFF_A[qi]
                            nc.gpsimd.tensor_tensor(
                                pt[:, :, 0:qn], pt[:, :, 0:qn],
                                erA_t[:, :, off:off + qn], mult)
                        pchunks = [pt[:, c, 0:qn] for c in range(KCH)]
                    else:
                        off = OFF_DC[qi]
                        pti = ptpool.tile([128, KCH, QC], I16, tag="ptd")
                        nc.vector.scalar_tensor_tensor(
                            pti[:, :, 0:qn], slab[:, :, 0:qn], B16,
                            rpDC_t[:, :, off:off + qn], add, add)
                        pchunks = [pti[:, c, 0:qn].bitcast(BF16)
                                   for c in range(KCH)]
                    pending.append((emit_pv, (qi, qn, pchunks)))
                    if len(pending) > 12:
                        fn, ent = pending.pop(0)
                        fn(ent)
                if b == 0:
                    # bg1's q-projection fills the phase-D ramp gaps
                    emit_q(1)
            for fn, ent in pending:
                fn(ent)

    return nc


def prep_host(inputs):
    f32 = np.float32
    bf = ml_dtypes.bfloat16
    f16 = np.float16
    f8 = ml_dtypes.float8_e4m3fn
    x = np.asarray(inputs["x"], f32)
    rpe = np.asarray(inputs["relative_pos_enc"], f32)
    q_w = np.asarray(inputs["q_w"], f32)[:, :, 0, 0]
    kv_w = np.asarray(inputs["kv_w"], f32)[:, :, 0, 0]
    kv_b = np.asarray(inputs["kv_b"], f32)
    sr1_w = np.asarray(inputs["sr1_w"], f32)[:, 0]
    lc_w = np.asarray(inputs["lc_w"], f32)[:, 0]
    lc_b = np.asarray(inputs["lc_b"], f32)
    eps = 1e-5

    a1 = np.asarray(inputs["sr1_gamma"], f32) / np.sqrt(
        np.asarray(inputs["sr1_var"], f32) + eps)
    b1 = np.asarray(inputs["sr1_beta"], f32) - np.asarray(
        inputs["sr1_mean"], f32) * a1
    aB2 = np.asarray(inputs["sr2_gamma"], f32) / np.sqrt(
        np.asarray(inputs["sr2_var"], f32) + eps)
    bB2 = np.asarray(inputs["sr2_beta"], f32) - np.asarray(
        inputs["sr2_mean"], f32) * aB2
    a2 = aB2 * np.asarray(inputs["sr2_w"], f32)[:, 0, 0, 0]
    c2 = bB2

    k9 = a2[:, None, None] * lc_w
    k9[:, 1, 1] += a2
    sv = np.zeros((C, HK, WK), f32)
    for i in range(3):
        for j in range(3):
            h0, h1 = max(0, 1 - i), min(HK, HK + 1 - i)
            w0, w1 = max(0, 1 - j), min(WK, WK + 1 - j)
            sv[:, h0:h1, w0:w1] += lc_w[:, i, j][:, None, None]
    const_map = c2[:, None] * (sv.reshape(C, NK) + 1.0) + lc_b[:, None]
    kv_const = kv_w @ const_map + kv_b[:, None]        # [2C, NK]
    assert np.allclose(np.asarray(inputs["q_b"], f32), 0)

    w25f = sr1_w.reshape(C, 25)
    w25d = np.zeros((C, 25, 128), f32)
    idx = np.arange(C)
    w25d[idx, :, idx % 128] = w25f
    w25d = w25d.astype(f8)
    w9d = np.zeros((C, 9, 128), f32)
    w9d[idx, :, idx % 128] = k9.reshape(C, 9)
    w9d = w9d.astype(bf)

    xp = np.zeros((B, C, 60, 60), f32)
    xp[:, :, 2:58, 2:58] = x

    idblk = np.zeros((128, 32), f32)
    for p in range(128):
        idblk[p, p % 32] = 1.0
    idblk = idblk.astype(bf)
    idn = np.eye(128, dtype=f32).astype(bf)

    xq_all = np.ascontiguousarray(x.reshape(B, C, NQ)).astype(f8)

    # kv_const chunks [4, 128, NK]
    kvc = np.ascontiguousarray(kv_const.reshape(4, 128, NK))

    # per-head R tables (columns packed by class)
    colsA = np.concatenate(
        [np.arange(qi * QC, qi * QC + _qn(qi)) for qi in range(N_QC)
         if MAP[qi] == 'A']) if W_A else np.zeros(0, np.int64)
    colsP = np.concatenate(
        [np.arange(qi * QC, qi * QC + _qn(qi)) for qi in range(N_QC)
         if MAP[qi] == 'P']) if W_P else np.zeros(0, np.int64)
    colsDC = np.concatenate(
        [np.arange(qi * QC, qi * QC + _qn(qi)) for qi in range(N_QC)
         if MAP[qi] in ('D', 'C')]) if W_DC else np.zeros(0, np.int64)

    in_maps = []
    for h in range(HEADS):
        Rt = np.zeros((NKP, NQ), f32)
        Rt[:NK, :] = rpe[0, h].T
        m = {
            "xp": np.ascontiguousarray(xp[h].reshape(C, 3600)).astype(f8),
            "xq": xq_all,
            "wqT": np.ascontiguousarray(
                (SCALE * A16 * q_w[h * 32:(h + 1) * 32]).T).astype(bf),
            "wkvT": np.ascontiguousarray(kv_w.T).astype(bf),
            "kvc": kvc,
            "w25d": w25d,
            "w9d": w9d,
            "ab1": np.ascontiguousarray(np.stack([a1, b1], 1)),
            "idblk": idblk,
            "idn": idn,
            "erA": np.ascontiguousarray(np.exp(Rt[:, colsA])).astype(bf),
            "rpeP": np.ascontiguousarray(A16 * Rt[:, colsP]).astype(bf),
            "rpeDC": np.ascontiguousarray(A16 * Rt[:, colsDC]).astype(f16),
        }
        in_maps.append(m)
    return in_maps


def kernel(**inputs):
    global LAST_RESULTS
    in_maps = prep_host(inputs)
    nc = bacc.Bacc("TRN2", target_bir_lowering=False, debug=False,
                   num_devices=HEADS)
    build(nc)
    nc.finalize()
    res = run_bass_kernel_spmd(
        nc, in_maps, core_ids=list(range(HEADS)),
        trace=bool(os.environ.get("KTRACE")))
    LAST_RESULTS = res

    po = np.array([(u // 15) * 512 + (u % 15) * 33 for u in range(N_QC)])
    cols = po[:, None] + np.arange(33)[None, :]        # [25, 33]
    out = np.empty((B, C, H, W), np.float32)
    for h in range(HEADS):
        o = res.results[h]["out"]                      # [B, 128, 1024] f32
        for b in range(B):
            blk = o[b][:, cols]                        # [128, 25, 33]
            flat = blk.transpose(1, 0, 2).reshape(-1, 33)[:NQ]
            out[b, h * 32:(h + 1) * 32] = (
                flat[:, :32] / flat[:, 32:33]).T.reshape(32, H, W)
    return out


# revision 12
# speedup vs baseline: 1.0002x; 1.0002x over previous
"""Head-parallel TRN2 kernel v2 for PVT-style spatial-reduction attention.

Core h owns head h for all 8 batches. Per-core phases:
  A: depthwise 5x5/s2 conv + BN/ReLU + folded 3x3 for OWN batch (PE block-diag)
  B: kv projection for OWN batch (all heads), const-add, k->fp8, v transposed;
     AllToAll redistributes (batch-sharded -> head-sharded), k fp8 / vT bf16
  Q: q projection, head-sharded (all batches), fp8 DoubleRow matmuls; q
     pre-scaled by SCALE*A16 so the S psum slab is a16*(S); DR-interleave via
     a DRAM bounce
  D: per (batch, 128-q-chunk) unit: S matmuls (fp8 DoubleRow) -> slab psum;
     exp via one of 4 paths (class map): ACT exp (+DVE er-mult or PE R-add),
     or fused Schraudolph bits on DVE/Pool (int16 -> bitcast bf16);
     PV with P^T chunk STATIONARY and vaug[k,33] moving -> out [q, 32+den].
  Normalization (divide by den) happens on host.
"""

import os
import sys
from contextlib import ExitStack

sys.path.insert(0, "/opt/trn_rl_repo")

import ml_dtypes
import numpy as np

import concourse.bass as bass
import concourse.mybir as mybir
import concourse.tile as tile
from concourse import bacc
from concourse.bass_utils import run_bass_kernel_spmd

F32 = mybir.dt.float32
F16 = mybir.dt.float16
BF16 = mybir.dt.bfloat16
FP8 = mybir.dt.float8e4
I16 = mybir.dt.int16

B, C, H, W = 8, 256, 56, 56
HEADS, SR, HD = 8, 2, 32
NQ = H * W            # 3136
HK, WK = H // SR, W // SR
NK = HK * WK          # 784
NKP = 896             # NK padded to 7*128
SCALE = HD ** -0.5
QC = 128
N_QC = (NQ + QC - 1) // QC   # 25 (last chunk 64 wide)
KCH = 7
A16 = 128.0 / np.log(2.0)    # schraudolph scale for bf16 bits
B16 = 16256.0 - 4.0          # schraudolph offset
PAYK = 32 * NK               # fp8 bytes of k per (batch, head)
PAYV = 128 * KCH * 32        # fp8 bytes of padded vT per (batch, head)
PAY = PAYK + PAYV

# per-qi class: A = ACT exp + Pool er-mult; P = PE R-add + ACT exp;
# D = DVE fused schraudolph (GPSIMD cannot read PSUM, so no Pool-fused path)
MAP = ['A', 'D', 'P', 'D', 'A', 'D', 'P', 'D', 'A', 'D', 'P', 'D', 'A',
       'D', 'P', 'D', 'P', 'D', 'P', 'D', 'A', 'D', 'A', 'P', 'P']
assert len(MAP) == N_QC


def _qn(qi):
    return min(QC, NQ - qi * QC)


def _offsets(classes):
    """column offset of each qi within the packed table for `classes`."""
    off, out = 0, {}
    for qi in range(N_QC):
        if MAP[qi] in classes:
            out[qi] = off
            off += _qn(qi)
    return out, off


OFF_A, W_A = _offsets(('A',))       # er = exp(R) cols, bf16
OFF_P, W_P = _offsets(('P',))       # a16*R cols, bf16
OFF_DC, W_DC = _offsets(('D',))     # a16*R cols, f16

LAST_RESULTS = None


def _kn(c):
    return 128 if c < KCH - 1 else NK - 128 * (KCH - 1)


def build(nc):
    mult = mybir.AluOpType.mult
    add = mybir.AluOpType.add
    DR = mybir.MatmulPerfMode.DoubleRow

    # ---- DRAM I/O ----
    xp_d = nc.dram_tensor("xp", [C, 60 * 60], FP8, kind="ExternalInput")
    xq_d = nc.dram_tensor("xq", [B, C, NQ], FP8, kind="ExternalInput")
    wq_d = nc.dram_tensor("wqT", [C, 32], BF16, kind="ExternalInput")
    wkv_d = nc.dram_tensor("wkvT", [C, 512], BF16, kind="ExternalInput")
    kvc_d = nc.dram_tensor("kvc", [4, 128, NK], F32, kind="ExternalInput")
    w25_d = nc.dram_tensor("w25d", [C, 25, 128], FP8, kind="ExternalInput")
    w9_d = nc.dram_tensor("w9d", [C, 9, 128], BF16, kind="ExternalInput")
    ab1_d = nc.dram_tensor("ab1", [C, 2], F32, kind="ExternalInput")
    idb_d = nc.dram_tensor("idblk", [128, 32], BF16, kind="ExternalInput")
    idn_d = nc.dram_tensor("idn", [128, 128], BF16, kind="ExternalInput")
    erA_d = nc.dram_tensor("erA", [NKP, W_A], BF16, kind="ExternalInput")
    rpP_d = nc.dram_tensor("rpeP", [NKP, W_P], BF16, kind="ExternalInput")
    rpDC_d = nc.dram_tensor("rpeDC", [NKP, W_DC], F16, kind="ExternalInput")
    out_d = nc.dram_tensor("out", [B, 128, 1024], F32, kind="ExternalOutput")

    # scratch + collective bounce
    qdr_d = nc.dram_tensor("qdr", [2, 128, NQ], FP8)
    a2a_in = nc.dram_tensor("a2a_in", [8, PAY], FP8)
    a2a_out = nc.dram_tensor("a2a_out", [8, PAY], FP8)

    with ExitStack() as ctx:
        tc = ctx.enter_context(tile.TileContext(nc))

        cpool = ctx.enter_context(tc.tile_pool(name="consts", bufs=1))
        wq_t = cpool.tile([128, 2, 32], BF16)
        wkv_t = cpool.tile([128, 2, 4, 128], BF16)
        kvc_t = cpool.tile([128, 4, NK], F32)
        idb_t = cpool.tile([128, 32], BF16)
        idn_t = cpool.tile([128, 128], BF16)
        erA_t = cpool.tile([128, KCH, W_A], BF16)
        rpP_t = cpool.tile([128, KCH, W_P], BF16)
        rpDC_t = cpool.tile([128, KCH, W_DC], F16)
        nc.sync.dma_start(idb_t[:], idb_d.ap())
        # (bulk R-table loads are emitted later, on the ACT queue, so
        # they don't compete with the conv/kv critical path for DMA engines)
        nc.scalar.dma_start(wq_t[:], wq_d.ap().rearrange(
            "(ch p) m -> p ch m", p=128))

        dpool = ctx.enter_context(tc.tile_pool(name="data", bufs=1))
        m_t = dpool.tile([128, 2, NK], BF16)
        kst_t = dpool.tile([128, 2, NK], FP8)     # k staging rows o*128+p
        vst_t = dpool.tile([128, 2, NK], BF16)    # v staging
        vtst_t = dpool.tile([128, KCH, 256], FP8)  # vT staging [k, (h d)]
        qf_t = dpool.tile([128, 2, NQ], FP8)      # q fp8, 4b x 32row layout
        q8_t = dpool.tile([128, 2, 2, NQ], FP8)   # DR layout, 4b x (16+16pad)
        k8_t = dpool.tile([128, 2, 2, NKP], FP8)
        vaug_t = dpool.tile([128, B, KCH, 33], FP8)
        nc.gpsimd.memset(k8_t[:], 0.0)
        nc.gpsimd.memset(vaug_t[:], 0.0)
        nc.gpsimd.memset(vaug_t[:, :, 0:KCH - 1, 32:33], 1.0)
        nc.gpsimd.memset(vaug_t[0:16, :, KCH - 1, 32:33], 1.0)

        xpool = ctx.enter_context(tc.tile_pool(name="xqP", bufs=5))
        xbs_all = []

        # ======== Phase A: conv for OWN batch ========
        with tc.tile_pool(name="convA", bufs=1) as apool, \
             tc.tile_pool(name="convPS", bufs=2, space="PSUM") as cps:
            w25_t = apool.tile([64, 2, 2, 25, 128], FP8)
            w9_t = apool.tile([128, 2, 9, 128], BF16)
            ab1_t = apool.tile([128, 2, 2], F32)
            xp_t = apool.tile([64, 2, 2, 60 * 60], FP8)
            tp_t = apool.tile([128, 2, 30 * 30], BF16)
            tmp = apool.tile([128, NK], F32, tag="tmp")
            nc.sync.dma_start(w25_t[:], w25_d.ap().rearrange(
                "(ch p two) t m -> p two ch t m", p=64, two=2))
            nc.sync.dma_start(w9_t[:], w9_d.ap().rearrange(
                "(c p) t m -> p c t m", p=128))
            nc.sync.dma_start(ab1_t[:], ab1_d.ap().rearrange(
                "(c p) m -> p c m", p=128))
            nc.sync.dma_start(
                xp_t[:], xp_d.ap().rearrange(
                    "(ch p two) n -> p two ch n", p=64, two=2))
            nc.sync.dma_start(wkv_t[:], wkv_d.ap().rearrange(
                "(ch p) (o m) -> p ch o m", p=128, m=128))
            nc.sync.dma_start(kvc_t[:], kvc_d.ap().rearrange(
                "o p n -> p o n"))
            for b in range(4):
                xb = xpool.tile([128, 2, NQ], FP8, tag="xb", name=f"xb{b}")
                nc.sync.dma_start(
                    xb[:], xq_d.ap()[b].rearrange(
                        "(ch p) n -> p ch n", p=128))
                xbs_all.append(xb)
            nc.gpsimd.memset(tp_t[:], 0.0)

            apss, mpss = [], []
            for ch in range(2):
                x5 = xp_t[:, :, ch, :].rearrange(
                    "p j (h s w t) -> p j h s w t", h=30, s=2, w=30, t=2)
                aps = cps.tile([128, 2, 512], F32, tag="cacc",
                               name=f"aps{ch}")
                apss.append(aps)
                for t in range(25):
                    i, j = divmod(t, 5)
                    qi_, ri = divmod(i, 2)
                    qj, rj = divmod(j, 2)
                    for nh, (r0, r1, nn) in enumerate(
                            ((0, 16, 448), (16, 28, 336))):
                        xv = x5[:, :, qi_ + r0:qi_ + r1, ri,
                                qj:qj + 28, rj]
                        nc.tensor.matmul(
                            aps[:, nh, 0:nn],
                            w25_t[:, :, ch, t, :],
                            xv, start=(t == 0), stop=(t == 24),
                            perf_mode=DR)
            for ch in range(2):
                tp3 = tp_t[:, ch, :].rearrange("p (h w) -> p h w", w=30)
                for nh, (r0, r1, nn) in enumerate(
                        ((0, 16, 448), (16, 28, 336))):
                    nc.vector.tensor_scalar(
                        tmp[:, 0:nn], apss[ch][:, nh, 0:nn],
                        ab1_t[:, ch, 0:1], ab1_t[:, ch, 1:2], mult, add)
                    nc.vector.tensor_scalar_max(
                        tp3[:, 1 + r0:1 + r1, 1:29],
                        tmp[:, 0:nn].rearrange("p (h w) -> p h w", w=28),
                        0.0)
            for ch in range(2):
                tp3 = tp_t[:, ch, :].rearrange("p (h w) -> p h w", w=30)
                mps = cps.tile([128, 2, 512], F32, tag="macc",
                               name=f"mps{ch}")
                mpss.append(mps)
                for t in range(9):
                    i, j = divmod(t, 3)
                    for nh, (r0, r1, nn) in enumerate(
                            ((0, 16, 448), (16, 28, 336))):
                        tpv = tp3[:, i + r0:i + r1, j:j + 28]
                        nc.tensor.matmul(
                            mps[:, nh, 0:nn],
                            w9_t[:, ch, t, :],
                            tpv, start=(t == 0), stop=(t == 8))
            for ch in range(2):
                for nh, (r0, r1, nn) in enumerate(
                        ((0, 16, 448), (16, 28, 336))):
                    nc.vector.tensor_copy(
                        m_t[:, ch, r0 * 28:r0 * 28 + nn],
                        mpss[ch][:, nh, 0:nn])

        # ======== Phase B: kv projection for OWN batch + AllToAll ========
        with tc.tile_pool(name="kvPS", bufs=2, space="PSUM") as kvps, \
             tc.tile_pool(name="vtPS", bufs=2, space="PSUM") as vtps:
            for o in range(4):   # out chunks: k0,k1,v0,v1
                ps = kvps.tile([128, 2, 512], F32, tag="kvp")
                dst = kst_t if o < 2 else vst_t
                for half, (h0, hn) in enumerate(((0, 448), (448, 336))):
                    for ch in range(2):
                        nc.tensor.matmul(
                            ps[:, half, 0:hn],
                            wkv_t[:, ch, o, :],
                            m_t[:, ch, h0:h0 + hn],
                            start=(ch == 0), stop=(ch == 1))
                    nc.vector.tensor_tensor(
                        dst[:, o % 2, h0:h0 + hn],
                        ps[:, half, 0:hn],
                        kvc_t[:, o, h0:h0 + hn], add)
                if o == 1:
                    # k fully staged: ship it while the v path computes
                    for hh in range(HEADS):
                        s2, o2 = hh % 4, hh // 4
                        nc.sync.dma_start(
                            a2a_in.ap()[hh, 0:PAYK].rearrange(
                                "(d n) -> d n", d=32),
                            kst_t[32 * s2:32 * s2 + 32, o2, :])
            # transpose v per (head, kchunk) -> vtst [k, 7, (h*32+d)]
            nc.vector.memset(vtst_t[:, KCH - 1, :], 0.0)
            for hh in range(HEADS):
                s, o = hh % 4, hh // 4
                vt = vtps.tile([128, KCH, 32], BF16, tag="vt")
                for c in range(KCH):
                    kn = _kn(c)
                    nc.tensor.transpose(
                        vt[0:kn, c, :],
                        vst_t[32 * s:32 * s + 32, o,
                              c * 128:c * 128 + kn],
                        idb_t[32 * s:32 * s + 32, :],
                        tile_position=(32 * s, 0))
                nc.scalar.copy(
                    vtst_t[:, 0:KCH - 1, 32 * hh:32 * hh + 32],
                    vt[:, 0:KCH - 1, :])
                nc.scalar.copy(
                    vtst_t[0:16, KCH - 1, 32 * hh:32 * hh + 32],
                    vt[0:16, KCH - 1, :])
            for hh in range(HEADS):
                nc.sync.dma_start(
                    a2a_in.ap()[hh, PAYK:PAY].rearrange(
                        "(c p d) -> p c d", c=KCH, p=128),
                    vtst_t[:, :, 32 * hh:32 * hh + 32])
            nc.gpsimd.collective_compute(
                "AllToAll",
                mybir.AluOpType.bypass,
                replica_groups=[list(range(8))],
                ins=[a2a_in.ap()],
                outs=[a2a_out.ap()],
            )

        # ======== Phases Q + D share one scope so they can overlap ========
        with tc.tile_pool(name="slabPS", bufs=3, space="PSUM") as spool, \
             tc.tile_pool(name="qPS", bufs=1, space="PSUM") as qpps, \
             tc.tile_pool(name="pvPS", bufs=1, space="PSUM") as pvpool, \
             tc.tile_pool(name="ptP", bufs=18) as ptpool, \
             tc.tile_pool(name="obP", bufs=2) as opool:
            # -------- Phase Q: q projection (4-batch col-tiled, fp8 in) ----
            def emit_q(bg):
                if bg == 0:
                    xbs = xbs_all
                else:
                    xbs = []
                    for bi in range(4):
                        b = bg * 4 + bi
                        xb = xpool.tile([128, 2, NQ], FP8, tag="xb")
                        nc.gpsimd.dma_start(
                            xb[:], xq_d.ap()[b].rearrange(
                                "(ch p) n -> p ch n", p=128))
                        xbs.append(xb)
                for nqi in range(7):
                    qps = qpps.tile([128, 448], F32, tag="qps")
                    for bi in range(4):
                        for ch in range(2):
                            nc.tensor.matmul(
                                qps[32 * bi:32 * bi + 32, :],
                                wq_t[:, ch, :],
                                xbs[bi][:, ch,
                                        nqi * 448:(nqi + 1) * 448],
                                start=(ch == 0), stop=(ch == 1),
                                tile_position=(0, 32 * bi))
                    nc.scalar.copy(
                        qf_t[:, bg, nqi * 448:(nqi + 1) * 448], qps[:])
                # bounce through DRAM to build the DR-interleaved layout
                nc.sync.dma_start(qdr_d.ap()[bg], qf_t[:, bg, :])
                for s in range(4):
                    nc.sync.dma_start(
                        q8_t[32 * s:32 * s + 16, bg, :, :],
                        qdr_d.ap()[bg, 32 * s:32 * s + 32, :].rearrange(
                            "(i j) n -> i j n", i=16))

            emit_q(0)

            # R tables + identity, needed only once phase D starts; the
            # gpsimd queue is dammed by the collective until staging is done
            nc.gpsimd.dma_start(idn_t[:], idn_d.ap())
            nc.gpsimd.dma_start(erA_t[:], erA_d.ap().rearrange(
                "(c p) w -> p c w", p=128))
            nc.gpsimd.dma_start(rpP_t[:], rpP_d.ap().rearrange(
                "(c p) w -> p c w", p=128))
            nc.gpsimd.dma_start(rpDC_t[:], rpDC_d.ap().rearrange(
                "(c p) w -> p c w", p=128))

            # k8/vaug loads (these wait on the collective; keep them after
            # the qdr bounce so they don't block the SP queue head)
            for b in range(B):
                bg, s = b // 4, b % 4
                nc.sync.dma_start(
                    k8_t[32 * s:32 * s + 16, bg, :, 0:NK],
                    a2a_out.ap()[b, 0:PAYK].rearrange(
                        "(i j n) -> i j n", i=16, j=2))
            for b in range(B):
                nc.sync.dma_start(
                    vaug_t[:, b, :, 0:32],
                    a2a_out.ap()[b, PAYK:PAY].rearrange(
                        "(c p d) -> p c d", c=KCH, p=128))

            # -------- Phase D: attention units --------
            pending = []
            for b in range(B):
                bg, s = b // 4, b % 4
                pvacc = pvpool.tile([128, 512], F32, tag="pv")
                ob = opool.tile([128, 1024], F32, tag="ob")

                def emit_pv(ent, b_=b, pv_=pvacc, ob_=ob):
                    qi_, qn_, pchunks_ = ent
                    po = (qi_ % 15) * 33
                    for c in range(KCH):
                        nc.tensor.matmul(
                            pv_[0:qn_, po:po + 33],
                            pchunks_[c],
                            vaug_t[:, b_, c, :],
                            start=(c == 0), stop=(c == KCH - 1))
                    # flush pv slots once exhausted (spread the WAR window)
                    if qi_ == 7:
                        nc.scalar.copy(ob_[:, 0:264], pv_[:, 0:264])
                    elif qi_ == 14:
                        nc.scalar.copy(ob_[:, 264:495], pv_[:, 264:495])
                    elif qi_ == 19:
                        nc.vector.tensor_copy(ob_[:, 512:677], pv_[:, 0:165])
                    elif qi_ == N_QC - 1:
                        nc.vector.tensor_copy(ob_[:, 677:842],
                                              pv_[:, 165:330])
                        nc.sync.dma_start(out_d.ap()[b_], ob_[:])

                for qi in range(N_QC):
                    q0, qn = qi * QC, _qn(qi)
                    cls = MAP[qi]
                    slab = spool.tile([128, KCH, QC], F32, tag="slab")
                    for c in range(KCH):
                        nc.tensor.matmul(
                            slab[:, c, 0:qn],
                            k8_t[32 * s:32 * s + 16, bg, :,
                                 c * 128:(c + 1) * 128],
                            q8_t[32 * s:32 * s + 16, bg, :, q0:q0 + qn],
                            start=True, stop=(cls != 'P'),
                            tile_position=(32 * s, 0),
                            perf_mode=DR)
                    if cls == 'P':
                        off = OFF_P[qi]
                        for c in range(KCH):
                            nc.tensor.matmul(
                                slab[:, c, 0:qn],
                                idn_t[:],
                                rpP_t[:, c, off:off + qn],
                                start=False, stop=True)
                    if cls in ('A', 'P'):
                        pt = ptpool.tile([128, KCH, QC], BF16, tag="pt")
                        nc.scalar.activation(
                            pt[:, :, 0:qn], slab[:, :, 0:qn],
                            mybir.ActivationFunctionType.Exp,
                            scale=float(1.0 / A16))
                        if cls == 'A':
                            off = OFF_A[qi]
                            nc.gpsimd.tensor_tensor(
                                pt[:, :, 0:qn], pt[:, :, 0:qn],
                                erA_t[:, :, off:off + qn], mult)
                        pchunks = [pt[:, c, 0:qn] for c in range(KCH)]
                    else:
                        off = OFF_DC[qi]
                        pti = ptpool.tile([128, KCH, QC], I16, tag="ptd")
                        nc.vector.scalar_tensor_tensor(
                            pti[:, :, 0:qn], slab[:, :, 0:qn], B16,
                            rpDC_t[:, :, off:off + qn], add, add)
                        pchunks = [pti[:, c, 0:qn].bitcast(BF16)
                                   for c in range(KCH)]
                    pending.append((emit_pv, (qi, qn, pchunks)))
                    if len(pending) > 12:
                        fn, ent = pending.pop(0)
                        fn(ent)
                if b == 0:
                    # bg1's q-projection fills the phase-D ramp gaps
                    emit_q(1)
            for fn, ent in pending:
                fn(ent)

    return nc


def prep_host(inputs):
    f32 = np.float32
    bf = ml_dtypes.bfloat16
    f16 = np.float16
    f8 = ml_dtypes.float8_e4m3fn
    x = np.asarray(inputs["x"], f32)
    rpe = np.asarray(inputs["relative_pos_enc"], f32)
    q_w = np.asarray(inputs["q_w"], f32)[:, :, 0, 0]
    kv_w = np.asarray(inputs["kv_w"], f32)[:, :, 0, 0]
    kv_b = np.asarray(inputs["kv_b"], f32)
    sr1_w = np.asarray(inputs["sr1_w"], f32)[:, 0]
    lc_w = np.asarray(inputs["lc_w"], f32)[:, 0]
    lc_b = np.asarray(inputs["lc_b"], f32)
    eps = 1e-5

    a1 = np.asarray(inputs["sr1_gamma"], f32) / np.sqrt(
        np.asarray(inputs["sr1_var"], f32) + eps)
    b1 = np.asarray(inputs["sr1_beta"], f32) - np.asarray(
        inputs["sr1_mean"], f32) * a1
    aB2 = np.asarray(inputs["sr2_gamma"], f32) / np.sqrt(
        np.asarray(inputs["sr2_var"], f32) + eps)
    bB2 = np.asarray(inputs["sr2_beta"], f32) - np.asarray(
        inputs["sr2_mean"], f32) * aB2
    a2 = aB2 * np.asarray(inputs["sr2_w"], f32)[:, 0, 0, 0]
    c2 = bB2

    k9 = a2[:, None, None] * lc_w
    k9[:, 1, 1] += a2
    sv = np.zeros((C, HK, WK), f32)
    for i in range(3):
        for j in range(3):
            h0, h1 = max(0, 1 - i), min(HK, HK + 1 - i)
            w0, w1 = max(0, 1 - j), min(WK, WK + 1 - j)
            sv[:, h0:h1, w0:w1] += lc_w[:, i, j][:, None, None]
    const_map = c2[:, None] * (sv.reshape(C, NK) + 1.0) + lc_b[:, None]
    kv_const = kv_w @ const_map + kv_b[:, None]        # [2C, NK]
    assert np.allclose(np.asarray(inputs["q_b"], f32), 0)

    w25f = sr1_w.reshape(C, 25)
    w25d = np.zeros((C, 25, 128), f32)
    idx = np.arange(C)
    w25d[idx, :, idx % 128] = w25f
    w25d = w25d.astype(f8)
    w9d = np.zeros((C, 9, 128), f32)
    w9d[idx, :, idx % 128] = k9.reshape(C, 9)
    w9d = w9d.astype(bf)

    xp = np.zeros((B, C, 60, 60), f32)
    xp[:, :, 2:58, 2:58] = x

    idblk = np.zeros((128, 32), f32)
    for p in range(128):
        idblk[p, p % 32] = 1.0
    idblk = idblk.astype(bf)
    idn = np.eye(128, dtype=f32).astype(bf)

    xq_all = np.ascontiguousarray(x.reshape(B, C, NQ)).astype(f8)

    # kv_const chunks [4, 128, NK]
    kvc = np.ascontiguousarray(kv_const.reshape(4, 128, NK))

    # per-head R tables (columns packed by class)
    colsA = np.concatenate(
        [np.arange(qi * QC, qi * QC + _qn(qi)) for qi in range(N_QC)
         if MAP[qi] == 'A']) if W_A else np.zeros(0, np.int64)
    colsP = np.concatenate(
        [np.arange(qi * QC, qi * QC + _qn(qi)) for qi in range(N_QC)
         if MAP[qi] == 'P']) if W_P else np.zeros(0, np.int64)
    colsDC = np.concatenate(
        [np.arange(qi * QC, qi * QC + _qn(qi)) for qi in range(N_QC)
         if MAP[qi] in ('D', 'C')]) if W_DC else np.zeros(0, np.int64)

    in_maps = []
    for h in range(HEADS):
        Rt = np.zeros((NKP, NQ), f32)
        Rt[:NK, :] = rpe[0, h].T
        m = {
            "xp": np.ascontiguousarray(xp[h].reshape(C, 3600)).astype(f8),
            "xq": xq_all,
            "wqT": np.ascontiguousarray(
                (SCALE * A16 * q_w[h * 32:(h + 1) * 32]).T).astype(bf),
            "wkvT": np.ascontiguousarray(kv_w.T).astype(bf),
            "kvc": kvc,
            "w25d": w25d,
            "w9d": w9d,
            "ab1": np.ascontiguousarray(np.stack([a1, b1], 1)),
            "idblk": idblk,
            "idn": idn,
            "erA": np.ascontiguousarray(np.exp(Rt[:, colsA])).astype(bf),
            "rpeP": np.ascontiguousarray(A16 * Rt[:, colsP]).astype(bf),
            "rpeDC": np.ascontiguousarray(A16 * Rt[:, colsDC]).astype(f16),
        }
        in_maps.append(m)
    return in_maps


def kernel(**inputs):
    global LAST_RESULTS
    in_maps = prep_host(inputs)
    nc = bacc.Bacc("TRN2", target_bir_lowering=False, debug=False,
                   num_devices=HEADS)
    build(nc)
    nc.finalize()
    res = run_bass_kernel_spmd(
        nc, in_maps, core_ids=list(range(HEADS)),
        trace=bool(os.environ.get("KTRACE")))
    LAST_RESULTS = res

    po = np.array([(u // 15) * 512 + (u % 15) * 33 for u in range(N_QC)])
    cols = po[:, None] + np.arange(33)[None, :]        # [25, 33]
    out = np.empty((B, C, H, W), np.float32)
    for h in range(HEADS):
        o = res.results[h]["out"]                      # [B, 128, 1024] f32
        for b in range(B):
            blk = o[b][:, cols]                        # [128, 25, 33]
            flat = blk.transpose(1, 0, 2).reshape(-1, 33)[:NQ]
            out[b, h * 32:(h + 1) * 32] = (
                flat[:, :32] / flat[:, 32:33]).T.reshape(32, H, W)
    return out


# revision 13
# speedup vs baseline: 1.0047x; 1.0045x over previous
"""Head-parallel TRN2 kernel v2 for PVT-style spatial-reduction attention.

Core h owns head h for all 8 batches. Per-core phases:
  A: depthwise 5x5/s2 conv + BN/ReLU + folded 3x3 for OWN batch (PE block-diag)
  B: kv projection for OWN batch (all heads), const-add, k->fp8, v transposed;
     AllToAll redistributes (batch-sharded -> head-sharded), k fp8 / vT bf16
  Q: q projection, head-sharded (all batches), fp8 DoubleRow matmuls; q
     pre-scaled by SCALE*A16 so the S psum slab is a16*(S); DR-interleave via
     a DRAM bounce
  D: per (batch, 128-q-chunk) unit: S matmuls (fp8 DoubleRow) -> slab psum;
     exp via one of 4 paths (class map): ACT exp (+DVE er-mult or PE R-add),
     or fused Schraudolph bits on DVE/Pool (int16 -> bitcast bf16);
     PV with P^T chunk STATIONARY and vaug[k,33] moving -> out [q, 32+den].
  Normalization (divide by den) happens on host.
"""

import os
import sys
from contextlib import ExitStack

sys.path.insert(0, "/opt/trn_rl_repo")

import ml_dtypes
import numpy as np

import concourse.bass as bass
import concourse.mybir as mybir
import concourse.tile as tile
from concourse import bacc
from concourse.bass_utils import run_bass_kernel_spmd

F32 = mybir.dt.float32
F16 = mybir.dt.float16
BF16 = mybir.dt.bfloat16
FP8 = mybir.dt.float8e4
I16 = mybir.dt.int16

B, C, H, W = 8, 256, 56, 56
HEADS, SR, HD = 8, 2, 32
NQ = H * W            # 3136
HK, WK = H // SR, W // SR
NK = HK * WK          # 784
NKP = 896             # NK padded to 7*128
SCALE = HD ** -0.5
QC = 128
N_QC = (NQ + QC - 1) // QC   # 25 (last chunk 64 wide)
KCH = 7
A16 = 128.0 / np.log(2.0)    # schraudolph scale for bf16 bits
B16 = 16256.0 - 4.0          # schraudolph offset
PAYK = 32 * NK               # fp8 bytes of k per (batch, head)
PAYV = 128 * KCH * 32        # fp8 bytes of padded vT per (batch, head)
PAY = PAYK + PAYV

# per-qi class: A = ACT exp + Pool er-mult; P = PE R-add + ACT exp;
# D = DVE fused schraudolph (GPSIMD cannot read PSUM, so no Pool-fused path)
MAP = ['A', 'D', 'P', 'D', 'A', 'D', 'P', 'D', 'A', 'D', 'P', 'D', 'A',
       'D', 'P', 'D', 'P', 'D', 'P', 'D', 'A', 'D', 'A', 'P', 'P']
assert len(MAP) == N_QC


def _qn(qi):
    return min(QC, NQ - qi * QC)


def _offsets(classes):
    """column offset of each qi within the packed table for `classes`."""
    off, out = 0, {}
    for qi in range(N_QC):
        if MAP[qi] in classes:
            out[qi] = off
            off += _qn(qi)
    return out, off


OFF_A, W_A = _offsets(('A',))       # er = exp(R) cols, bf16
OFF_P, W_P = _offsets(('P',))       # a16*R cols, bf16
OFF_DC, W_DC = _offsets(('D',))     # a16*R cols, f16

LAST_RESULTS = None


def _kn(c):
    return 128 if c < KCH - 1 else NK - 128 * (KCH - 1)


def build(nc):
    mult = mybir.AluOpType.mult
    add = mybir.AluOpType.add
    DR = mybir.MatmulPerfMode.DoubleRow

    # ---- DRAM I/O ----
    xp_d = nc.dram_tensor("xp", [C, 60 * 60], FP8, kind="ExternalInput")
    xq_d = nc.dram_tensor("xq", [B, C, NQ], FP8, kind="ExternalInput")
    wq_d = nc.dram_tensor("wqT", [C, 32], BF16, kind="ExternalInput")
    wkv_d = nc.dram_tensor("wkvT", [C, 512], BF16, kind="ExternalInput")
    kvc_d = nc.dram_tensor("kvc", [4, 128, NK], F32, kind="ExternalInput")
    w25_d = nc.dram_tensor("w25d", [C, 25, 128], FP8, kind="ExternalInput")
    w9_d = nc.dram_tensor("w9d", [C, 9, 128], BF16, kind="ExternalInput")
    ab1_d = nc.dram_tensor("ab1", [C, 2], F32, kind="ExternalInput")
    idb_d = nc.dram_tensor("idblk", [128, 32], BF16, kind="ExternalInput")
    idn_d = nc.dram_tensor("idn", [128, 128], BF16, kind="ExternalInput")
    erA_d = nc.dram_tensor("erA", [NKP, W_A], BF16, kind="ExternalInput")
    rpP_d = nc.dram_tensor("rpeP", [NKP, W_P], BF16, kind="ExternalInput")
    rpDC_d = nc.dram_tensor("rpeDC", [NKP, W_DC], F16, kind="ExternalInput")
    out_d = nc.dram_tensor("out", [B, 128, 1024], F32, kind="ExternalOutput")

    # scratch + collective bounce
    qdr_d = nc.dram_tensor("qdr", [2, 128, NQ], FP8)
    a2a_in = nc.dram_tensor("a2a_in", [8, PAY], FP8)
    a2a_out = nc.dram_tensor("a2a_out", [8, PAY], FP8)

    with ExitStack() as ctx:
        tc = ctx.enter_context(tile.TileContext(nc))

        cpool = ctx.enter_context(tc.tile_pool(name="consts", bufs=1))
        wq_t = cpool.tile([128, 2, 32], BF16)
        wkv_t = cpool.tile([128, 2, 4, 128], BF16)
        kvc_t = cpool.tile([128, 4, NK], F32)
        idb_t = cpool.tile([128, 32], BF16)
        idn_t = cpool.tile([128, 128], BF16)
        erA_t = cpool.tile([128, KCH, W_A], BF16)
        rpP_t = cpool.tile([128, KCH, W_P], BF16)
        rpDC_t = cpool.tile([128, KCH, W_DC], F16)
        nc.sync.dma_start(idb_t[:], idb_d.ap())
        # (bulk R-table loads are emitted later, on the ACT queue, so
        # they don't compete with the conv/kv critical path for DMA engines)
        nc.scalar.dma_start(wq_t[:], wq_d.ap().rearrange(
            "(ch p) m -> p ch m", p=128))

        dpool = ctx.enter_context(tc.tile_pool(name="data", bufs=1))
        m_t = dpool.tile([128, 2, NK], BF16)
        kst_t = dpool.tile([128, 2, NK], FP8)     # k staging rows o*128+p
        vst_t = dpool.tile([128, 2, NK], BF16)    # v staging
        vtst_t = dpool.tile([128, KCH, 256], FP8)  # vT staging [k, (h d)]
        qf_t = dpool.tile([128, 2, NQ], FP8)      # q fp8, 4b x 32row layout
        q8_t = dpool.tile([128, 2, 2, NQ], FP8)   # DR layout, 4b x (16+16pad)
        k8_t = dpool.tile([128, 2, 2, NKP], FP8)
        vaug_t = dpool.tile([128, B, KCH, 33], FP8)
        nc.gpsimd.memset(k8_t[:], 0.0)
        nc.gpsimd.memset(vaug_t[:], 0.0)
        nc.gpsimd.memset(vaug_t[:, :, 0:KCH - 1, 32:33], 1.0)
        nc.gpsimd.memset(vaug_t[0:16, :, KCH - 1, 32:33], 1.0)

        xpool = ctx.enter_context(tc.tile_pool(name="xqP", bufs=5))
        xbs_all = []

        # ======== Phase A: conv for OWN batch ========
        with tc.tile_pool(name="convA", bufs=1) as apool, \
             tc.tile_pool(name="convPS", bufs=2, space="PSUM") as cps:
            w25_t = apool.tile([64, 2, 2, 25, 128], FP8)
            w9_t = apool.tile([128, 2, 9, 128], BF16)
            ab1_t = apool.tile([128, 2, 2], F32)
            xp_t = apool.tile([64, 2, 2, 60 * 60], FP8)
            tp_t = apool.tile([128, 2, 30 * 30], BF16)
            tmp = apool.tile([128, NK], F32, tag="tmp")
            nc.sync.dma_start(w25_t[:], w25_d.ap().rearrange(
                "(ch p two) t m -> p two ch t m", p=64, two=2))
            nc.sync.dma_start(w9_t[:], w9_d.ap().rearrange(
                "(c p) t m -> p c t m", p=128))
            nc.sync.dma_start(ab1_t[:], ab1_d.ap().rearrange(
                "(c p) m -> p c m", p=128))
            nc.sync.dma_start(
                xp_t[:], xp_d.ap().rearrange(
                    "(ch p two) n -> p two ch n", p=64, two=2))
            nc.sync.dma_start(wkv_t[:], wkv_d.ap().rearrange(
                "(ch p) (o m) -> p ch o m", p=128, m=128))
            nc.sync.dma_start(kvc_t[:], kvc_d.ap().rearrange(
                "o p n -> p o n"))
            for b in range(4):
                xb = xpool.tile([128, 2, NQ], FP8, tag="xb", name=f"xb{b}")
                nc.sync.dma_start(
                    xb[:], xq_d.ap()[b].rearrange(
                        "(ch p) n -> p ch n", p=128))
                xbs_all.append(xb)
            nc.gpsimd.memset(tp_t[:], 0.0)

            apss, mpss = [], []
            for ch in range(2):
                x5 = xp_t[:, :, ch, :].rearrange(
                    "p j (h s w t) -> p j h s w t", h=30, s=2, w=30, t=2)
                aps = cps.tile([128, 2, 512], F32, tag="cacc",
                               name=f"aps{ch}")
                apss.append(aps)
                for t in range(25):
                    i, j = divmod(t, 5)
                    qi_, ri = divmod(i, 2)
                    qj, rj = divmod(j, 2)
                    for nh, (r0, r1, nn) in enumerate(
                            ((0, 16, 448), (16, 28, 336))):
                        xv = x5[:, :, qi_ + r0:qi_ + r1, ri,
                                qj:qj + 28, rj]
                        nc.tensor.matmul(
                            aps[:, nh, 0:nn],
                            w25_t[:, :, ch, t, :],
                            xv, start=(t == 0), stop=(t == 24),
                            perf_mode=DR)
            for ch in range(2):
                tp3 = tp_t[:, ch, :].rearrange("p (h w) -> p h w", w=30)
                for nh, (r0, r1, nn) in enumerate(
                        ((0, 16, 448), (16, 28, 336))):
                    nc.vector.tensor_scalar(
                        tmp[:, 0:nn], apss[ch][:, nh, 0:nn],
                        ab1_t[:, ch, 0:1], ab1_t[:, ch, 1:2], mult, add)
                    nc.vector.tensor_scalar_max(
                        tp3[:, 1 + r0:1 + r1, 1:29],
                        tmp[:, 0:nn].rearrange("p (h w) -> p h w", w=28),
                        0.0)
            for ch in range(2):
                tp3 = tp_t[:, ch, :].rearrange("p (h w) -> p h w", w=30)
                mps = cps.tile([128, 2, 512], F32, tag="macc",
                               name=f"mps{ch}")
                mpss.append(mps)
                for t in range(9):
                    i, j = divmod(t, 3)
                    for nh, (r0, r1, nn) in enumerate(
                            ((0, 16, 448), (16, 28, 336))):
                        tpv = tp3[:, i + r0:i + r1, j:j + 28]
                        nc.tensor.matmul(
                            mps[:, nh, 0:nn],
                            w9_t[:, ch, t, :],
                            tpv, start=(t == 0), stop=(t == 8))
            for ch in range(2):
                for nh, (r0, r1, nn) in enumerate(
                        ((0, 16, 448), (16, 28, 336))):
                    nc.vector.tensor_copy(
                        m_t[:, ch, r0 * 28:r0 * 28 + nn],
                        mpss[ch][:, nh, 0:nn])

        # ======== Phase B: kv projection for OWN batch + AllToAll ========
        with tc.tile_pool(name="kvPS", bufs=2, space="PSUM") as kvps, \
             tc.tile_pool(name="vtPS", bufs=4, space="PSUM") as vtps:
            for o in range(4):   # out chunks: k0,k1,v0,v1
                ps = kvps.tile([128, 2, 512], F32, tag="kvp")
                dst = kst_t if o < 2 else vst_t
                for half, (h0, hn) in enumerate(((0, 448), (448, 336))):
                    for ch in range(2):
                        nc.tensor.matmul(
                            ps[:, half, 0:hn],
                            wkv_t[:, ch, o, :],
                            m_t[:, ch, h0:h0 + hn],
                            start=(ch == 0), stop=(ch == 1))
                    nc.vector.tensor_tensor(
                        dst[:, o % 2, h0:h0 + hn],
                        ps[:, half, 0:hn],
                        kvc_t[:, o, h0:h0 + hn], add)
                if o == 1:
                    # k fully staged: ship it while the v path computes
                    for hh in range(HEADS):
                        s2, o2 = hh % 4, hh // 4
                        nc.sync.dma_start(
                            a2a_in.ap()[hh, 0:PAYK].rearrange(
                                "(d n) -> d n", d=32),
                            kst_t[32 * s2:32 * s2 + 32, o2, :])
            # transpose v per (head, kchunk) -> vtst [k, 7, (h*32+d)]
            nc.vector.memset(vtst_t[:, KCH - 1, :], 0.0)
            for hh in range(HEADS):
                s, o = hh % 4, hh // 4
                vt = vtps.tile([128, KCH, 32], BF16, tag="vt")
                for c in range(KCH):
                    kn = _kn(c)
                    nc.tensor.transpose(
                        vt[0:kn, c, :],
                        vst_t[32 * s:32 * s + 32, o,
                              c * 128:c * 128 + kn],
                        idb_t[32 * s:32 * s + 32, :],
                        tile_position=(32 * s, 0))
                ceng = nc.vector.tensor_copy if hh < 4 else nc.scalar.copy
                ceng(vtst_t[:, 0:KCH - 1, 32 * hh:32 * hh + 32],
                     vt[:, 0:KCH - 1, :])
                ceng(vtst_t[0:16, KCH - 1, 32 * hh:32 * hh + 32],
                     vt[0:16, KCH - 1, :])
            for hh in range(HEADS):
                nc.sync.dma_start(
                    a2a_in.ap()[hh, PAYK:PAY].rearrange(
                        "(c p d) -> p c d", c=KCH, p=128),
                    vtst_t[:, :, 32 * hh:32 * hh + 32])
            nc.gpsimd.collective_compute(
                "AllToAll",
                mybir.AluOpType.bypass,
                replica_groups=[list(range(8))],
                ins=[a2a_in.ap()],
                outs=[a2a_out.ap()],
            )

        # ======== Phases Q + D share one scope so they can overlap ========
        with tc.tile_pool(name="slabPS", bufs=3, space="PSUM") as spool, \
             tc.tile_pool(name="qPS", bufs=1, space="PSUM") as qpps, \
             tc.tile_pool(name="pvPS", bufs=1, space="PSUM") as pvpool, \
             tc.tile_pool(name="ptP", bufs=18) as ptpool, \
             tc.tile_pool(name="obP", bufs=2) as opool:
            # -------- Phase Q: q projection (4-batch col-tiled, fp8 in) ----
            def emit_q(bg):
                if bg == 0:
                    xbs = xbs_all
                else:
                    xbs = []
                    for bi in range(4):
                        b = bg * 4 + bi
                        xb = xpool.tile([128, 2, NQ], FP8, tag="xb")
                        nc.gpsimd.dma_start(
                            xb[:], xq_d.ap()[b].rearrange(
                                "(ch p) n -> p ch n", p=128))
                        xbs.append(xb)
                for nqi in range(7):
                    qps = qpps.tile([128, 448], F32, tag="qps")
                    for bi in range(4):
                        for ch in range(2):
                            nc.tensor.matmul(
                                qps[32 * bi:32 * bi + 32, :],
                                wq_t[:, ch, :],
                                xbs[bi][:, ch,
                                        nqi * 448:(nqi + 1) * 448],
                                start=(ch == 0), stop=(ch == 1),
                                tile_position=(0, 32 * bi))
                    nc.scalar.copy(
                        qf_t[:, bg, nqi * 448:(nqi + 1) * 448], qps[:])
                # bounce through DRAM to build the DR-interleaved layout
                nc.sync.dma_start(qdr_d.ap()[bg], qf_t[:, bg, :])
                for s in range(4):
                    nc.sync.dma_start(
                        q8_t[32 * s:32 * s + 16, bg, :, :],
                        qdr_d.ap()[bg, 32 * s:32 * s + 32, :].rearrange(
                            "(i j) n -> i j n", i=16))

            emit_q(0)

            # R tables + identity, needed only once phase D starts; the
            # gpsimd queue is dammed by the collective until staging is done
            nc.gpsimd.dma_start(idn_t[:], idn_d.ap())
            nc.gpsimd.dma_start(erA_t[:], erA_d.ap().rearrange(
                "(c p) w -> p c w", p=128))
            nc.gpsimd.dma_start(rpP_t[:], rpP_d.ap().rearrange(
                "(c p) w -> p c w", p=128))
            nc.gpsimd.dma_start(rpDC_t[:], rpDC_d.ap().rearrange(
                "(c p) w -> p c w", p=128))

            # k8/vaug loads (these wait on the collective; keep them after
            # the qdr bounce so they don't block the SP queue head)
            for b in range(B):
                bg, s = b // 4, b % 4
                nc.sync.dma_start(
                    k8_t[32 * s:32 * s + 16, bg, :, 0:NK],
                    a2a_out.ap()[b, 0:PAYK].rearrange(
                        "(i j n) -> i j n", i=16, j=2))
            for b in range(B):
                nc.sync.dma_start(
                    vaug_t[:, b, :, 0:32],
                    a2a_out.ap()[b, PAYK:PAY].rearrange(
                        "(c p d) -> p c d", c=KCH, p=128))

            # -------- Phase D: attention units --------
            pending = []
            for b in range(B):
                bg, s = b // 4, b % 4
                pvacc = pvpool.tile([128, 512], F32, tag="pv")
                ob = opool.tile([128, 1024], F32, tag="ob")

                def emit_pv(ent, b_=b, pv_=pvacc, ob_=ob):
                    qi_, qn_, pchunks_ = ent
                    po = (qi_ % 15) * 33
                    for c in range(KCH):
                        nc.tensor.matmul(
                            pv_[0:qn_, po:po + 33],
                            pchunks_[c],
                            vaug_t[:, b_, c, :],
                            start=(c == 0), stop=(c == KCH - 1))
                    # flush pv slots once exhausted (spread the WAR window)
                    if qi_ == 7:
                        nc.scalar.copy(ob_[:, 0:264], pv_[:, 0:264])
                    elif qi_ == 14:
                        nc.scalar.copy(ob_[:, 264:495], pv_[:, 264:495])
                    elif qi_ == 19:
                        nc.vector.tensor_copy(ob_[:, 512:677], pv_[:, 0:165])
                    elif qi_ == N_QC - 1:
                        nc.vector.tensor_copy(ob_[:, 677:842],
                                              pv_[:, 165:330])
                        nc.sync.dma_start(out_d.ap()[b_], ob_[:])

                for qi in range(N_QC):
                    q0, qn = qi * QC, _qn(qi)
                    cls = MAP[qi]
                    slab = spool.tile([128, KCH, QC], F32, tag="slab")
                    for c in range(KCH):
                        nc.tensor.matmul(
                            slab[:, c, 0:qn],
                            k8_t[32 * s:32 * s + 16, bg, :,
                                 c * 128:(c + 1) * 128],
                            q8_t[32 * s:32 * s + 16, bg, :, q0:q0 + qn],
                            start=True, stop=(cls != 'P'),
                            tile_position=(32 * s, 0),
                            perf_mode=DR)
                    if cls == 'P':
                        off = OFF_P[qi]
                        for c in range(KCH):
                            nc.tensor.matmul(
                                slab[:, c, 0:qn],
                                idn_t[:],
                                rpP_t[:, c, off:off + qn],
                                start=False, stop=True)
                    if cls in ('A', 'P'):
                        pt = ptpool.tile([128, KCH, QC], BF16, tag="pt")
                        nc.scalar.activation(
                            pt[:, :, 0:qn], slab[:, :, 0:qn],
                            mybir.ActivationFunctionType.Exp,
                            scale=float(1.0 / A16))
                        if cls == 'A':
                            off = OFF_A[qi]
                            nc.gpsimd.tensor_tensor(
                                pt[:, :, 0:qn], pt[:, :, 0:qn],
                                erA_t[:, :, off:off + qn], mult)
                        pchunks = [pt[:, c, 0:qn] for c in range(KCH)]
                    else:
                        off = OFF_DC[qi]
                        pti = ptpool.tile([128, KCH, QC], I16, tag="ptd")
                        nc.vector.scalar_tensor_tensor(
                            pti[:, :, 0:qn], slab[:, :, 0:qn], B16,
                            rpDC_t[:, :, off:off + qn], add, add)
                        pchunks = [pti[:, c, 0:qn].bitcast(BF16)
                                   for c in range(KCH)]
                    pending.append((emit_pv, (qi, qn, pchunks)))
                    if len(pending) > 12:
                        fn, ent = pending.pop(0)
                        fn(ent)
                if b == 0:
                    # bg1's q-projection fills the phase-D ramp gaps
                    emit_q(1)
            for fn, ent in pending:
                fn(ent)

    return nc


def prep_host(inputs):
    f32 = np.float32
    bf = ml_dtypes.bfloat16
    f16 = np.float16
    f8 = ml_dtypes.float8_e4m3fn
    x = np.asarray(inputs["x"], f32)
    rpe = np.asarray(inputs["relative_pos_enc"], f32)
    q_w = np.asarray(inputs["q_w"], f32)[:, :, 0, 0]
    kv_w = np.asarray(inputs["kv_w"], f32)[:, :, 0, 0]
    kv_b = np.asarray(inputs["kv_b"], f32)
    sr1_w = np.asarray(inputs["sr1_w"], f32)[:, 0]
    lc_w = np.asarray(inputs["lc_w"], f32)[:, 0]
    lc_b = np.asarray(inputs["lc_b"], f32)
    eps = 1e-5

    a1 = np.asarray(inputs["sr1_gamma"], f32) / np.sqrt(
        np.asarray(inputs["sr1_var"], f32) + eps)
    b1 = np.asarray(inputs["sr1_beta"], f32) - np.asarray(
        inputs["sr1_mean"], f32) * a1
    aB2 = np.asarray(inputs["sr2_gamma"], f32) / np.sqrt(
        np.asarray(inputs["sr2_var"], f32) + eps)
    bB2 = np.asarray(inputs["sr2_beta"], f32) - np.asarray(
        inputs["sr2_mean"], f32) * aB2
    a2 = aB2 * np.asarray(inputs["sr2_w"], f32)[:, 0, 0, 0]
    c2 = bB2

    k9 = a2[:, None, None] * lc_w
    k9[:, 1, 1] += a2
    sv = np.zeros((C, HK, WK), f32)
    for i in range(3):
        for j in range(3):
            h0, h1 = max(0, 1 - i), min(HK, HK + 1 - i)
            w0, w1 = max(0, 1 - j), min(WK, WK + 1 - j)
            sv[:, h0:h1, w0:w1] += lc_w[:, i, j][:, None, None]
    const_map = c2[:, None] * (sv.reshape(C, NK) + 1.0) + lc_b[:, None]
    kv_const = kv_w @ const_map + kv_b[:, None]        # [2C, NK]
    assert np.allclose(np.asarray(inputs["q_b"], f32), 0)

    w25f = sr1_w.reshape(C, 25)
    w25d = np.zeros((C, 25, 128), f32)
    idx = np.arange(C)
    w25d[idx, :, idx % 128] = w25f
    w25d = w25d.astype(f8)
    w9d = np.zeros((C, 9, 128), f32)
    w9d[idx, :, idx % 128] = k9.reshape(C, 9)
    w9d = w9d.astype(bf)

    xp = np.zeros((B, C, 60, 60), f32)
    xp[:, :, 2:58, 2:58] = x

    idblk = np.zeros((128, 32), f32)
    for p in range(128):
        idblk[p, p % 32] = 1.0
    idblk = idblk.astype(bf)
    idn = np.eye(128, dtype=f32).astype(bf)

    xq_all = np.ascontiguousarray(x.reshape(B, C, NQ)).astype(f8)

    # kv_const chunks [4, 128, NK]
    kvc = np.ascontiguousarray(kv_const.reshape(4, 128, NK))

    # per-head R tables (columns packed by class)
    colsA = np.concatenate(
        [np.arange(qi * QC, qi * QC + _qn(qi)) for qi in range(N_QC)
         if MAP[qi] == 'A']) if W_A else np.zeros(0, np.int64)
    colsP = np.concatenate(
        [np.arange(qi * QC, qi * QC + _qn(qi)) for qi in range(N_QC)
         if MAP[qi] == 'P']) if W_P else np.zeros(0, np.int64)
    colsDC = np.concatenate(
        [np.arange(qi * QC, qi * QC + _qn(qi)) for qi in range(N_QC)
         if MAP[qi] in ('D', 'C')]) if W_DC else np.zeros(0, np.int64)

    in_maps = []
    for h in range(HEADS):
        Rt = np.zeros((NKP, NQ), f32)
        Rt[:NK, :] = rpe[0, h].T
        m = {
            "xp": np.ascontiguousarray(xp[h].reshape(C, 3600)).astype(f8),
            "xq": xq_all,
            "wqT": np.ascontiguousarray(
                (SCALE * A16 * q_w[h * 32:(h + 1) * 32]).T).astype(bf),
            "wkvT": np.ascontiguousarray(kv_w.T).astype(bf),
            "kvc": kvc,
            "w25d": w25d,
            "w9d": w9d,
            "ab1": np.ascontiguousarray(np.stack([a1, b1], 1)),
            "idblk": idblk,
            "idn": idn,
            "erA": np.ascontiguousarray(np.exp(Rt[:, colsA])).astype(bf),
            "rpeP": np.ascontiguousarray(A16 * Rt[:, colsP]).astype(bf),
            "rpeDC": np.ascontiguousarray(A16 * Rt[:, colsDC]).astype(f16),
        }
        in_maps.append(m)
    return in_maps


def kernel(**inputs):
    global LAST_RESULTS
    in_maps = prep_host(inputs)
    nc = bacc.Bacc("TRN2", target_bir_lowering=False, debug=False,
                   num_devices=HEADS)
    build(nc)
    nc.finalize()
    res = run_bass_kernel_spmd(
        nc, in_maps, core_ids=list(range(HEADS)),
        trace=bool(os.environ.get("KTRACE")))
    LAST_RESULTS = res

    po = np.array([(u // 15) * 512 + (u % 15) * 33 for u in range(N_QC)])
    cols = po[:, None] + np.arange(33)[None, :]        # [25, 33]
    out = np.empty((B, C, H, W), np.float32)
    for h in range(HEADS):
        o = res.results[h]["out"]                      # [B, 128, 1024] f32
        for b in range(B):
            blk = o[b][:, cols]                        # [128, 25, 33]
            flat = blk.transpose(1, 0, 2).reshape(-1, 33)[:NQ]
            out[b, h * 32:(h + 1) * 32] = (
                flat[:, :32] / flat[:, 32:33]).T.reshape(32, H, W)
    return out
